# revision 44
# baseline (speedup 1.0000x reference)
"""MultiHeadGAT layer on 8 Trainium2 NeuronCores — v3 (gather-free streams).

Strategy (graph/data parallel, dst-sharded, per sharding hint):
  - Nodes partitioned into 8 ranges (6250/core); each core owns its output
    rows.  Edges routed host-side to the core/tile owning their destination,
    padded to 128-edge chunks.  All params + x replicated; no collectives.
  - No node table and no dma_gather: the host streams x^T with columns
    repeated in edge-slot order (a pure permutation/replication of the
    input), so every DMA is a big contiguous HWDGE transfer.  Per chunk one
    matmul  lhsT=x_jT[128k,128lane] @ rhsBT[128k, 132]  produces
    xl_j = W_lin x_j (128 cols) and aj = xl_j . att_dst (4 cols) in PSUM.
  - alpha = lrelu(aj + OHdt^T s_own + (ea*C16) tree-sum);  e = exp(alpha)
    (no segment-max shift; alphas bounded);  msg = e * xl_j with channels
    stored (c,h)-interleaved so the DVE runs its 2x bf16 mode (e is read
    along the innermost stride-1 head axis).  One-hot matmuls accumulate
    [numerator | denom] per tile in PSUM, software-pipelined one tile
    behind the attention math so the PE never waits on msg; the
    epilogue's first multiply un-interleaves back to (h,c) order free.
  - Epilogue batched 8 tiles wide: divide, +bias, +residual, LayerNorm
    (rstd via exp(-0.5 ln(var+eps))), ELU.
"""

import math

import numpy as np

import concourse.bass as bass
import concourse.bacc as bacc
import concourse.mybir as mybir
from concourse.tile import TileContext
from concourse.masks import make_identity
from concourse.bass_utils import run_bass_kernel_spmd

F32 = mybir.dt.float32
BF16 = mybir.dt.bfloat16
FP8 = mybir.dt.float8e4
U8 = mybir.dt.uint8
AF = mybir.ActivationFunctionType
OP = mybir.AluOpType
AX = mybir.AxisListType

H, C = 4, 32
HC = H * C          # 128
IN_CH = 128
ED = 16
NEG_SLOPE = 0.2
LN_EPS = 1e-5
P = 128
T_EP = 12           # tiles per epilogue batch
G3 = 3              # chunks per ps_x psum bank (3*132 <= 512)

N_NODES = 50000
N_CORES = 8
NPC = N_NODES // N_CORES          # 6250
TILES = math.ceil(NPC / P)        # 49
NPAD = TILES * P                  # 6272

ONE_FP8 = 0x38  # 1.0 in float8_e4m3


# --------------------------------------------------------------------------
# host-side routing (index bookkeeping + layout only)
# --------------------------------------------------------------------------

def host_prep(x, edge_index, edge_attr, W_lin, W_edge, att_src, att_dst,
              att_edge, bias, ln_gamma, ln_beta):
    import ml_dtypes
    bf = ml_dtypes.bfloat16

    src = np.asarray(edge_index[0], np.int64)
    dst = np.asarray(edge_index[1], np.int64)
    ea = np.asarray(edge_attr, np.float32)
    E = src.shape[0]

    core_of = dst // NPC
    local = dst - core_of * NPC
    tile_of = local >> 7
    rel = local & 127

    key = core_of * TILES + tile_of
    order = np.argsort(key, kind="stable")
    key_s = key[order]
    counts = np.bincount(key_s, minlength=N_CORES * TILES)
    M = max(1, int(math.ceil(counts.max() / P)))
    C_TOT = TILES * M

    group_start = np.zeros(N_CORES * TILES, np.int64)
    np.cumsum(counts[:-1], out=group_start[1:])
    rank = np.arange(E, dtype=np.int64) - group_start[key_s]

    t_s = tile_of[order]
    c_s = core_of[order]
    rel_s = rel[order]
    src_s = src[order]
    ea_s = ea[order]

    chunk = t_s * M + (rank >> 7)
    lane = rank & 127

    x = np.asarray(x, np.float32)
    xTbf = np.ascontiguousarray(x.T).astype(bf)   # [128, N]

    # (c,h)-interleaved channel order: permuted row r' = c*H + h holds
    # original row h*C + c.  With W_lin/att rows permuted this way, the
    # per-chunk matmul emits xl already (c,h)-interleaved (what the DVE 2x
    # msg multiply wants) and all dot-products over channels are unchanged.
    rp = np.arange(HC)
    perm = (rp % H) * C + rp // H   # perm[c*H+h] = h*C + c
    wl = np.ascontiguousarray(np.asarray(W_lin, np.float32)[perm]).astype(bf)
    we = np.ascontiguousarray(np.asarray(W_edge, np.float32)[perm]).astype(bf)
    a_src = np.asarray(att_src, np.float32).reshape(HC)[perm]
    a_dst = np.asarray(att_dst, np.float32).reshape(HC)[perm]
    a_edge = np.asarray(att_edge, np.float32).reshape(HC)[perm]
    # block-diagonal attention matrix [HC, 3H] in permuted row space:
    # cols 0:H att_dst (aj, src side), H:2H att_src (s_own, dst side),
    # 2H:3H att_edge.  Pure placement of input values.
    a_bd = np.zeros((HC, 3 * H), np.float32)
    rows = np.arange(HC)
    heads = rows % H         # head of permuted row c*H+h is h
    a_bd[rows, heads] = a_dst
    a_bd[rows, H + heads] = a_src
    a_bd[rows, 2 * H + heads] = a_edge
    a_bd = a_bd.astype(bf)
    bias_r = np.asarray(bias, np.float32).reshape(1, HC)
    gamma_r = np.asarray(ln_gamma, np.float32).reshape(1, HC)
    beta_r = np.asarray(ln_beta, np.float32).reshape(1, HC)

    in_maps = []
    for c in range(N_CORES):
        m = c_s == c
        ch = chunk[m]
        ln = lane[m]
        rl = rel_s[m]
        sc = src_s[m]
        slot = ch * P + ln

        # x^T replicated into edge-slot order (pad slots -> 0)
        xjT = np.zeros((IN_CH, C_TOT * P), bf)
        xjT[:, slot] = xTbf[:, sc]

        # partition-major one-hots: [P, C_TOT, P]
        oh = np.zeros((C_TOT, P, P), np.uint8)
        oh[ch, ln, rl] = ONE_FP8
        oh_pm = np.ascontiguousarray(oh.transpose(1, 0, 2))
        ohdt = np.ascontiguousarray(oh.transpose(2, 0, 1))

        eat = np.zeros((C_TOT, P, ED), np.float32)
        eat[ch, ln] = ea_s[m]
        eat = np.ascontiguousarray(eat.transpose(1, 0, 2)).astype(bf)

        n0 = c * NPC
        xres = np.zeros((NPAD, IN_CH), np.float32)
        xres[:NPC] = x[n0:n0 + NPC]
        xresT = np.ascontiguousarray(xres.T).astype(bf)

        in_maps.append(dict(
            xjT=xjT,
            xresT=xresT,
            xres=xres,
            oh=oh_pm,
            ohdt=ohdt,
            ea_sw=eat,
            wl=wl,
            we=we,
            a_bd=a_bd,
            bias=bias_r,
            ln_gamma=gamma_r,
            ln_beta=beta_r,
        ))
    return in_maps, M


# --------------------------------------------------------------------------
# device program
# --------------------------------------------------------------------------

def build_program(M, num_devices=None):
    C_TOT = TILES * M

    nc = bacc.Bacc("TRN2", target_bir_lowering=False, debug=False,
                   num_devices=num_devices or N_CORES)

    dp = nc.declare_dram_parameter
    xjT_d = dp("xjT", [IN_CH, C_TOT * P], BF16, isOutput=False)
    xresT_d = dp("xresT", [IN_CH, NPAD], BF16, isOutput=False)
    xres_d = dp("xres", [NPAD, IN_CH], F32, isOutput=False)
    oh_d = dp("oh", [P, C_TOT, P], U8, isOutput=False)
    ohdt_d = dp("ohdt", [P, C_TOT, P], U8, isOutput=False)
    ea_d = dp("ea_sw", [P, C_TOT, ED], BF16, isOutput=False)
    wl_d = dp("wl", [HC, IN_CH], BF16, isOutput=False)
    we_d = dp("we", [HC, ED], BF16, isOutput=False)
    abd_d = dp("a_bd", [HC, 3 * H], BF16, isOutput=False)
    bias_d = dp("bias", [1, HC], F32, isOutput=False)
    gamma_d = dp("ln_gamma", [1, HC], F32, isOutput=False)
    beta_d = dp("ln_beta", [1, HC], F32, isOutput=False)
    out_d = dp("out", [NPAD, HC], F32, isOutput=True)

    with TileContext(nc) as tc:
        with (
            tc.tile_pool(name="const", bufs=1) as cpool,
            tc.tile_pool(name="stream", bufs=2) as spool,
            tc.tile_pool(name="work", bufs=2) as wpool,
            tc.tile_pool(name="ep", bufs=2) as epool,
            tc.tile_pool(name="ps_a", bufs=1, space="PSUM") as papool,
            tc.tile_pool(name="ps_x", bufs=3, space="PSUM") as pxpool,
            tc.tile_pool(name="ps_al", bufs=2, space="PSUM") as plpool,
            tc.tile_pool(name="ps_acc", bufs=2, space="PSUM") as pcpool,
        ):
            # ---------------- phase A: constants --------------------------
            ident = cpool.tile([P, P], BF16, tag="ident")
            make_identity(nc, ident[:])

            wl_sb = cpool.tile([HC, IN_CH], BF16, tag="wl")
            nc.sync.dma_start(out=wl_sb[:], in_=wl_d[:])
            we_sb = cpool.tile([HC, ED], BF16, tag="we")
            nc.sync.dma_start(out=we_sb[:], in_=we_d[:])
            # host-built block-diagonal attention matrix (permuted row space)
            a_bd = cpool.tile([HC, 3 * H], BF16, tag="a_bd")
            nc.sync.dma_start(out=a_bd[:], in_=abd_d[:])

            # rhsBT [in_ch, 136] = [ W_lin^T | B_dst(aj) | B_src(s_own) ]
            rhsBT = cpool.tile([IN_CH, HC + 2 * H], BF16, tag="rhsbt")
            wlT_ps = papool.tile([P, P], BF16, tag="psA")
            nc.tensor.transpose(out=wlT_ps[:], in_=wl_sb[:], identity=ident[:])
            nc.scalar.copy(out=rhsBT[:, 0:HC], in_=wlT_ps[:])
            b8_ps = papool.tile([IN_CH, 2 * H], F32, tag="psA")
            nc.tensor.matmul(out=b8_ps[:], lhsT=wl_sb[:],
                             rhs=a_bd[:, 0:2 * H], start=True, stop=True)
            nc.vector.tensor_copy(out=rhsBT[:, HC:HC + 2 * H], in_=b8_ps[:])

            c16_ps = papool.tile([ED, H], F32, tag="psA")
            nc.tensor.matmul(out=c16_ps[:], lhsT=we_sb[:],
                             rhs=a_bd[:, 2 * H:3 * H], start=True, stop=True)
            c16 = cpool.tile([ED, H], BF16, tag="c16")
            nc.vector.tensor_copy(out=c16[:], in_=c16_ps[:])
            # c16T [H, ED] -> broadcast [P, H, ED] for the DVE ae product
            c16t_ps = papool.tile([H, ED], BF16, tag="psA")
            nc.tensor.transpose(out=c16t_ps[:], in_=c16[:],
                                identity=ident[0:ED, 0:ED])
            c16t = cpool.tile([H, ED], BF16, tag="c16t")
            nc.vector.tensor_copy(out=c16t[:], in_=c16t_ps[:])
            c16t_dram = nc.dram_tensor("c16t_scratch", [H, ED], BF16)
            nc.sync.dma_start(out=c16t_dram[:], in_=c16t[:])
            c16b = cpool.tile([P, H, ED], BF16, tag="c16b")
            nc.sync.dma_start(
                out=c16b[:],
                in_=c16t_dram[:].rearrange("a b -> (a b)")
                    .unsqueeze(0).to_broadcast([P, H * ED]))

            bias_b = cpool.tile([P, HC], F32, tag="bias_b")
            nc.sync.dma_start(out=bias_b[:], in_=bias_d[:].to_broadcast([P, HC]))
            gamma_b = cpool.tile([P, HC], F32, tag="gamma_b")
            nc.sync.dma_start(out=gamma_b[:],
                              in_=gamma_d[:].to_broadcast([P, HC]))
            beta_b = cpool.tile([P, HC], F32, tag="beta_b")
            nc.sync.dma_start(out=beta_b[:], in_=beta_d[:].to_broadcast([P, HC]))

            eps_t = cpool.tile([P, 1], F32, tag="eps_t")
            nc.gpsimd.memset(eps_t[:], LN_EPS)
            tiny_t = cpool.tile([P, 1], F32, tag="tiny_t")
            nc.gpsimd.memset(tiny_t[:], 1e-16)

            # xrb = residual + bias, precomputed once
            xrb_sb = cpool.tile([P, TILES, HC], F32, tag="xrb")
            nc.sync.dma_start(
                out=xrb_sb[:],
                in_=xres_d[:].rearrange("(t p) c -> p t c", p=P))
            nc.vector.tensor_tensor(
                out=xrb_sb[:], in0=xrb_sb[:],
                in1=bias_b[:].unsqueeze(1).to_broadcast([P, TILES, HC]),
                op=OP.add)

            # s_own [128, TILES*H] bf16 (xl . att_src for own nodes)
            xresT_sb = cpool.tile([IN_CH, NPAD], BF16, tag="xresT")
            nc.sync.dma_start(out=xresT_sb[:], in_=xresT_d[:])
            s_own = cpool.tile([P, TILES * H], BF16, tag="s_own")
            for t in range(TILES):
                so_ps = papool.tile([P, H], F32, tag="psA")
                nc.tensor.matmul(out=so_ps[:],
                                 lhsT=xresT_sb[:, t * P:(t + 1) * P],
                                 rhs=rhsBT[:, HC + H:HC + 2 * H],
                                 start=True, stop=True)
                nc.vector.tensor_copy(out=s_own[:, t * H:(t + 1) * H],
                                      in_=so_ps[:])

            # ---------------- phase C: edges (per dst tile) ---------------
            stage_ep = None
            ep_fill = 0
            ep_base = 0

            def flush_epilogue(stage_ep, n_tiles, t0):
                # stage_ep: [P, T_EP, HC+H] f32, tiles t0..t0+n_tiles-1.
                # num cols 0:HC are (c,h)-interleaved; the first multiply
                # below restores standard (h,c) order via a strided read.
                nt = n_tiles
                num = stage_ep[:, :nt, 0:HC]
                den = stage_ep[:, :nt, HC:HC + H]
                rden = epool.tile([P, T_EP, H], F32, tag="rden")
                nc.scalar.activation(out=rden[:, :nt, :], in_=den,
                                     func=AF.Identity, bias=tiny_t[:, 0:1])
                nc.vector.reciprocal(out=rden[:, :nt, :], in_=rden[:, :nt, :])
                o = epool.tile([P, T_EP, HC], F32, tag="o")
                nc.vector.tensor_tensor(
                    out=o[:, :nt, :].rearrange("p t (h c) -> p t h c", c=C),
                    in0=num.rearrange("p t (c h) -> p t h c", h=H),
                    in1=rden[:, :nt, :].unsqueeze(3)
                        .to_broadcast([P, nt, H, C]),
                    op=OP.mult)
                nc.vector.tensor_tensor(out=o[:, :nt, :], in0=o[:, :nt, :],
                                        in1=xrb_sb[:, t0:t0 + nt, :],
                                        op=OP.add)
                # LayerNorm across channels
                mu = epool.tile([P, T_EP], F32, tag="mu")
                nc.vector.reduce_sum(out=mu[:, :nt], in_=o[:, :nt, :],
                                     axis=AX.X)
                nc.scalar.mul(out=mu[:, :nt], in_=mu[:, :nt], mul=1.0 / HC)
                nc.vector.tensor_tensor(
                    out=o[:, :nt, :], in0=o[:, :nt, :],
                    in1=mu[:, :nt].unsqueeze(2).to_broadcast([P, nt, HC]),
                    op=OP.subtract)
                sq = epool.tile([P, T_EP, HC], F32, tag="sq")
                nc.vector.tensor_tensor(out=sq[:, :nt, :], in0=o[:, :nt, :],
                                        in1=o[:, :nt, :], op=OP.mult)
                var = epool.tile([P, T_EP], F32, tag="var")
                nc.vector.reduce_sum(out=var[:, :nt], in_=sq[:, :nt, :],
                                     axis=AX.X)
                # rstd = exp(-0.5 * ln(var/HC + eps))
                nc.scalar.activation(out=var[:, :nt], in_=var[:, :nt],
                                     func=AF.Ln, scale=1.0 / HC,
                                     bias=eps_t[:, 0:1])
                nc.scalar.activation(out=var[:, :nt], in_=var[:, :nt],
                                     func=AF.Exp, scale=-0.5)
                nc.vector.tensor_tensor(
                    out=o[:, :nt, :], in0=o[:, :nt, :],
                    in1=var[:, :nt].unsqueeze(2).to_broadcast([P, nt, HC]),
                    op=OP.mult)
                nc.vector.tensor_tensor(
                    out=o[:, :nt, :], in0=o[:, :nt, :],
                    in1=gamma_b[:].unsqueeze(1).to_broadcast([P, nt, HC]),
                    op=OP.mult)
                nc.vector.tensor_tensor(
                    out=o[:, :nt, :], in0=o[:, :nt, :],
                    in1=beta_b[:].unsqueeze(1).to_broadcast([P, nt, HC]),
                    op=OP.add)
                # ELU = relu(x) + min(exp(x)-1, 0)
                ex = epool.tile([P, T_EP, HC], F32, tag="ex")
                nc.scalar.activation(out=ex[:, :nt, :], in_=o[:, :nt, :],
                                     func=AF.Exp)
                nc.vector.tensor_scalar(out=ex[:, :nt, :], in0=ex[:, :nt, :],
                                        scalar1=-1.0, scalar2=0.0,
                                        op0=OP.add, op1=OP.min)
                nc.scalar.activation(out=o[:, :nt, :], in_=o[:, :nt, :],
                                     func=AF.Relu)
                nc.vector.tensor_tensor(out=o[:, :nt, :], in0=o[:, :nt, :],
                                        in1=ex[:, :nt, :], op=OP.add)
                nc.sync.dma_start(
                    out=out_d[t0 * P:(t0 + nt) * P, :]
                        .rearrange("(t p) c -> p t c", p=P),
                    in_=o[:, :nt, :])

            def emit_acc(msg_t, oh_t, t):
                # accumulate [numerator | denom] for tile t (deferred one
                # tile so the PE never stalls waiting for msg)
                nonlocal stage_ep, ep_fill, ep_base
                acc = pcpool.tile([P, HC + H], F32, name="acc_t", tag="acc")
                for cidx in range(M):
                    nc.tensor.matmul(
                        out=acc[:],
                        lhsT=oh_t[:, cidx, :].bitcast(FP8),
                        rhs=msg_t[:, cidx, :],
                        start=(cidx == 0), stop=(cidx == M - 1))
                if ep_fill == 0:
                    stage_ep = epool.tile([P, T_EP, HC + H], F32,
                                          tag="stage_ep")
                    ep_base = t
                nc.scalar.copy(out=stage_ep[:, t - ep_base, :],
                               in_=acc[:])
                ep_fill += 1
                if ep_fill == T_EP or t == TILES - 1:
                    flush_epilogue(stage_ep, ep_fill, ep_base)
                    ep_fill = 0

            pend = None
            for t in range(TILES):
                cb = t * M

                xjT_sb = spool.tile([IN_CH, M * P], BF16, tag="xjt")
                nc.sync.dma_start(out=xjT_sb[:],
                                  in_=xjT_d[:, cb * P:(cb + M) * P])
                oh_sb = spool.tile([P, M, P], U8, tag="oh")
                nc.sync.dma_start(out=oh_sb[:], in_=oh_d[:, cb:cb + M, :])
                ohdt_sb = spool.tile([P, M, P], U8, tag="ohdt")
                nc.sync.dma_start(out=ohdt_sb[:], in_=ohdt_d[:, cb:cb + M, :])
                ea_sb = spool.tile([P, M, ED], BF16, tag="ea")
                nc.sync.dma_start(out=ea_sb[:], in_=ea_d[:, cb:cb + M, :])

                # ae = (ea * C16t) tree-summed over ED on DVE (bf16, 2x) —
                # emitted first so the DVE starts as soon as ea_sb lands.
                prod = wpool.tile([P, M, H, ED], BF16, tag="prod")
                nc.vector.tensor_tensor(
                    out=prod[:],
                    in0=ea_sb[:].unsqueeze(2).to_broadcast([P, M, H, ED]),
                    in1=c16b[:].unsqueeze(1).to_broadcast([P, M, H, ED]),
                    op=OP.mult)
                for w in (8, 4, 2, 1):
                    nc.vector.tensor_tensor(
                        out=prod[:, :, :, 0:w],
                        in0=prod[:, :, :, 0:w],
                        in1=prod[:, :, :, w:2 * w],
                        op=OP.add)

                # sdst = OHdt^T s_own  (per chunk, 4 cols) — emitted before
                # mm_x so the DVE can fold ae+sdst while ACT copies run.
                al_ps = plpool.tile([P, M * H], F32, tag="al")
                for cidx in range(M):
                    sl = slice(cidx * H, (cidx + 1) * H)
                    nc.tensor.matmul(
                        out=al_ps[:, sl],
                        lhsT=ohdt_sb[:, cidx, :].bitcast(FP8),
                        rhs=s_own[:, t * H:(t + 1) * H],
                        start=True, stop=True, skip_group_check=True)

                # alpha-partial = ae + sdst (aj not ready yet)
                alpha = wpool.tile([P, M, H], F32, tag="alpha")
                nc.vector.tensor_tensor(
                    out=alpha[:],
                    in0=prod[:, :, :, 0],
                    in1=al_ps[:].rearrange("p (m h) -> p m h", h=H),
                    op=OP.add)

                # xl_j | aj per chunk via PE (already (c,h)-interleaved —
                # W_lin rows were permuted host-side), one merged ACT copy.
                xj_sb = wpool.tile([P, M, HC + H], BF16, tag="xj")
                for g0 in range(0, M, G3):
                    k = min(G3, M - g0)
                    psx = pxpool.tile([P, G3, HC + H], F32, tag="psx")
                    for i in range(k):
                        nc.tensor.matmul(
                            out=psx[:, i, :],
                            lhsT=xjT_sb[:, (g0 + i) * P:(g0 + i + 1) * P],
                            rhs=rhsBT[:, 0:HC + H],
                            start=True, stop=True)
                    nc.scalar.copy(out=xj_sb[:, g0:g0 + k, :],
                                   in_=psx[:, :k, :])

                # previous tile's accumulation goes here on the PE queue:
                # its msg is ready by now, and it fills the PE pipeline
                # while this tile's alpha/exp/msg run on ACT/DVE.
                if pend is not None:
                    emit_acc(*pend)

                # alpha = lrelu(alpha-partial + aj)
                nc.vector.tensor_tensor(
                    out=alpha[:],
                    in0=alpha[:],
                    in1=xj_sb[:, :, HC:HC + H],
                    op=OP.add)
                nc.vector.scalar_tensor_tensor(
                    out=alpha[:], in0=alpha[:],
                    scalar=NEG_SLOPE, in1=alpha[:],
                    op0=OP.mult, op1=OP.max)

                # msg = e * xl  ((c,h)-interleaved, DVE 2x bf16 mode)
                msg = wpool.tile([P, M, HC + H], BF16, tag="msg")
                nc.scalar.activation(out=msg[:, :, HC:HC + H],
                                     in_=alpha[:], func=AF.Exp)
                nc.vector.tensor_tensor(
                    out=msg[:, :, 0:HC].rearrange("p m (c h) -> p m c h", h=H),
                    in0=xj_sb[:, :, 0:HC]
                        .rearrange("p m (c h) -> p m c h", h=H),
                    in1=msg[:, :, HC:HC + H].unsqueeze(2)
                        .to_broadcast([P, M, C, H]),
                    op=OP.mult)

                pend = (msg, oh_sb, t)
            emit_acc(*pend)

    nc.compile()
    return nc


# --------------------------------------------------------------------------
# entry point
# --------------------------------------------------------------------------

def kernel(**inputs) -> np.ndarray:
    in_maps, M = host_prep(**inputs)
    nc = build_program(M)
    res = run_bass_kernel_spmd(nc, in_maps, list(range(N_CORES)))
    parts = [res.results[c]["out"][:NPC] for c in range(N_CORES)]
    return np.concatenate(parts, axis=0).astype(np.float32)


# revision 46
# speedup vs baseline: 1.0212x; 1.0212x over previous
"""MultiHeadGAT layer on 8 Trainium2 NeuronCores — v3 (gather-free streams).

Strategy (graph/data parallel, dst-sharded, per sharding hint):
  - Nodes partitioned into 8 ranges (6250/core); each core owns its output
    rows.  Edges routed host-side to the core/tile owning their destination,
    padded to 128-edge chunks.  All params + x replicated; no collectives.
  - No node table and no dma_gather: the host streams x^T with columns
    repeated in edge-slot order (a pure permutation/replication of the
    input), so every DMA is a big contiguous HWDGE transfer.  Per chunk one
    matmul  lhsT=x_jT[128k,128lane] @ rhsBT[128k, 132]  produces
    xl_j = W_lin x_j (128 cols) and aj = xl_j . att_dst (4 cols) in PSUM.
  - alpha = lrelu(aj + OHdt^T s_own + (ea*C16) tree-sum);  e = exp(alpha)
    (no segment-max shift; alphas bounded);  msg = e * xl_j with channels
    stored (c,h)-interleaved so the DVE runs its 2x bf16 mode (e is read
    along the innermost stride-1 head axis).  One-hot matmuls accumulate
    [numerator | denom] per tile in PSUM, software-pipelined one tile
    behind the attention math so the PE never waits on msg; the
    epilogue's first multiply un-interleaves back to (h,c) order free.
  - Epilogue batched 8 tiles wide: divide, +bias, +residual, LayerNorm
    (rstd via exp(-0.5 ln(var+eps))), ELU.
"""

import math

import numpy as np

import concourse.bass as bass
import concourse.bacc as bacc
import concourse.mybir as mybir
from concourse.tile import TileContext
from concourse.masks import make_identity
from concourse.bass_utils import run_bass_kernel_spmd

F32 = mybir.dt.float32
BF16 = mybir.dt.bfloat16
FP8 = mybir.dt.float8e4
U8 = mybir.dt.uint8
AF = mybir.ActivationFunctionType
OP = mybir.AluOpType
AX = mybir.AxisListType

H, C = 4, 32
HC = H * C          # 128
IN_CH = 128
ED = 16
NEG_SLOPE = 0.2
LN_EPS = 1e-5
P = 128
T_EP = 8            # tiles per epilogue batch
G3 = 3              # chunks per ps_x psum bank (3*132 <= 512)

N_NODES = 50000
N_CORES = 8
NPC = N_NODES // N_CORES          # 6250
TILES = math.ceil(NPC / P)        # 49
NPAD = TILES * P                  # 6272

ONE_FP8 = 0x38  # 1.0 in float8_e4m3


# --------------------------------------------------------------------------
# host-side routing (index bookkeeping + layout only)
# --------------------------------------------------------------------------

def host_prep(x, edge_index, edge_attr, W_lin, W_edge, att_src, att_dst,
              att_edge, bias, ln_gamma, ln_beta):
    import ml_dtypes
    bf = ml_dtypes.bfloat16

    src = np.asarray(edge_index[0], np.int64)
    dst = np.asarray(edge_index[1], np.int64)
    ea = np.asarray(edge_attr, np.float32)
    E = src.shape[0]

    core_of = dst // NPC
    local = dst - core_of * NPC
    tile_of = local >> 7
    rel = local & 127

    key = core_of * TILES + tile_of
    order = np.argsort(key, kind="stable")
    key_s = key[order]
    counts = np.bincount(key_s, minlength=N_CORES * TILES)
    M = max(1, int(math.ceil(counts.max() / P)))
    C_TOT = TILES * M

    group_start = np.zeros(N_CORES * TILES, np.int64)
    np.cumsum(counts[:-1], out=group_start[1:])
    rank = np.arange(E, dtype=np.int64) - group_start[key_s]

    t_s = tile_of[order]
    c_s = core_of[order]
    rel_s = rel[order]
    src_s = src[order]
    ea_s = ea[order]

    chunk = t_s * M + (rank >> 7)
    lane = rank & 127

    x = np.asarray(x, np.float32)
    xTbf = np.ascontiguousarray(x.T).astype(bf)   # [128, N]

    # (c,h)-interleaved channel order: permuted row r' = c*H + h holds
    # original row h*C + c.  With W_lin/att rows permuted this way, the
    # per-chunk matmul emits xl already (c,h)-interleaved (what the DVE 2x
    # msg multiply wants) and all dot-products over channels are unchanged.
    rp = np.arange(HC)
    perm = (rp % H) * C + rp // H   # perm[c*H+h] = h*C + c
    wl = np.ascontiguousarray(np.asarray(W_lin, np.float32)[perm]).astype(bf)
    we = np.ascontiguousarray(np.asarray(W_edge, np.float32)[perm]).astype(bf)
    a_src = np.asarray(att_src, np.float32).reshape(HC)[perm]
    a_dst = np.asarray(att_dst, np.float32).reshape(HC)[perm]
    a_edge = np.asarray(att_edge, np.float32).reshape(HC)[perm]
    # block-diagonal attention matrix [HC, 3H] in permuted row space:
    # cols 0:H att_dst (aj, src side), H:2H att_src (s_own, dst side),
    # 2H:3H att_edge.  Pure placement of input values.
    a_bd = np.zeros((HC, 3 * H), np.float32)
    rows = np.arange(HC)
    heads = rows % H         # head of permuted row c*H+h is h
    a_bd[rows, heads] = a_dst
    a_bd[rows, H + heads] = a_src
    a_bd[rows, 2 * H + heads] = a_edge
    a_bd = a_bd.astype(bf)
    bias_r = np.asarray(bias, np.float32).reshape(1, HC)
    gamma_r = np.asarray(ln_gamma, np.float32).reshape(1, HC)
    beta_r = np.asarray(ln_beta, np.float32).reshape(1, HC)

    in_maps = []
    for c in range(N_CORES):
        m = c_s == c
        ch = chunk[m]
        ln = lane[m]
        rl = rel_s[m]
        sc = src_s[m]
        slot = ch * P + ln

        # x^T replicated into edge-slot order (pad slots -> 0)
        xjT = np.zeros((IN_CH, C_TOT * P), bf)
        xjT[:, slot] = xTbf[:, sc]

        # partition-major one-hots: [P, C_TOT, P]
        oh = np.zeros((C_TOT, P, P), np.uint8)
        oh[ch, ln, rl] = ONE_FP8
        oh_pm = np.ascontiguousarray(oh.transpose(1, 0, 2))
        ohdt = np.ascontiguousarray(oh.transpose(2, 0, 1))

        eat = np.zeros((C_TOT, P, ED), np.float32)
        eat[ch, ln] = ea_s[m]
        eat = np.ascontiguousarray(eat.transpose(1, 0, 2)).astype(bf)

        n0 = c * NPC
        xres = np.zeros((NPAD, IN_CH), np.float32)
        xres[:NPC] = x[n0:n0 + NPC]
        xresT = np.ascontiguousarray(xres.T).astype(bf)

        in_maps.append(dict(
            xjT=xjT,
            xresT=xresT,
            xres=xres,
            oh=oh_pm,
            ohdt=ohdt,
            ea_sw=eat,
            wl=wl,
            we=we,
            a_bd=a_bd,
            bias=bias_r,
            ln_gamma=gamma_r,
            ln_beta=beta_r,
        ))
    return in_maps, M


# --------------------------------------------------------------------------
# device program
# --------------------------------------------------------------------------

def build_program(M, num_devices=None):
    C_TOT = TILES * M

    nc = bacc.Bacc("TRN2", target_bir_lowering=False, debug=False,
                   num_devices=num_devices or N_CORES)

    dp = nc.declare_dram_parameter
    xjT_d = dp("xjT", [IN_CH, C_TOT * P], BF16, isOutput=False)
    xresT_d = dp("xresT", [IN_CH, NPAD], BF16, isOutput=False)
    xres_d = dp("xres", [NPAD, IN_CH], F32, isOutput=False)
    oh_d = dp("oh", [P, C_TOT, P], U8, isOutput=False)
    ohdt_d = dp("ohdt", [P, C_TOT, P], U8, isOutput=False)
    ea_d = dp("ea_sw", [P, C_TOT, ED], BF16, isOutput=False)
    wl_d = dp("wl", [HC, IN_CH], BF16, isOutput=False)
    we_d = dp("we", [HC, ED], BF16, isOutput=False)
    abd_d = dp("a_bd", [HC, 3 * H], BF16, isOutput=False)
    bias_d = dp("bias", [1, HC], F32, isOutput=False)
    gamma_d = dp("ln_gamma", [1, HC], F32, isOutput=False)
    beta_d = dp("ln_beta", [1, HC], F32, isOutput=False)
    out_d = dp("out", [NPAD, HC], F32, isOutput=True)

    with TileContext(nc) as tc:
        with (
            tc.tile_pool(name="const", bufs=1) as cpool,
            tc.tile_pool(name="stream", bufs=2) as spool,
            tc.tile_pool(name="work", bufs=2) as wpool,
            tc.tile_pool(name="ep", bufs=2) as epool,
            tc.tile_pool(name="ps_a", bufs=1, space="PSUM") as papool,
            tc.tile_pool(name="ps_x", bufs=3, space="PSUM") as pxpool,
            tc.tile_pool(name="ps_al", bufs=2, space="PSUM") as plpool,
            tc.tile_pool(name="ps_acc", bufs=2, space="PSUM") as pcpool,
        ):
            # ---------------- phase A: constants --------------------------
            ident = cpool.tile([P, P], BF16, tag="ident")
            make_identity(nc, ident[:])

            wl_sb = cpool.tile([HC, IN_CH], BF16, tag="wl")
            nc.sync.dma_start(out=wl_sb[:], in_=wl_d[:])
            we_sb = cpool.tile([HC, ED], BF16, tag="we")
            nc.sync.dma_start(out=we_sb[:], in_=we_d[:])
            # host-built block-diagonal attention matrix (permuted row space)
            a_bd = cpool.tile([HC, 3 * H], BF16, tag="a_bd")
            nc.sync.dma_start(out=a_bd[:], in_=abd_d[:])

            # rhsBT [in_ch, 136] = [ W_lin^T | B_dst(aj) | B_src(s_own) ]
            rhsBT = cpool.tile([IN_CH, HC + 2 * H], BF16, tag="rhsbt")
            wlT_ps = papool.tile([P, P], BF16, tag="psA")
            nc.tensor.transpose(out=wlT_ps[:], in_=wl_sb[:], identity=ident[:])
            nc.scalar.copy(out=rhsBT[:, 0:HC], in_=wlT_ps[:])
            b8_ps = papool.tile([IN_CH, 2 * H], F32, tag="psA")
            nc.tensor.matmul(out=b8_ps[:], lhsT=wl_sb[:],
                             rhs=a_bd[:, 0:2 * H], start=True, stop=True)
            nc.vector.tensor_copy(out=rhsBT[:, HC:HC + 2 * H], in_=b8_ps[:])

            c16_ps = papool.tile([ED, H], F32, tag="psA")
            nc.tensor.matmul(out=c16_ps[:], lhsT=we_sb[:],
                             rhs=a_bd[:, 2 * H:3 * H], start=True, stop=True)
            c16 = cpool.tile([ED, H], BF16, tag="c16")
            nc.vector.tensor_copy(out=c16[:], in_=c16_ps[:])
            # c16T [H, ED] -> broadcast [P, H, ED] for the DVE ae product
            c16t_ps = papool.tile([H, ED], BF16, tag="psA")
            nc.tensor.transpose(out=c16t_ps[:], in_=c16[:],
                                identity=ident[0:ED, 0:ED])
            c16t = cpool.tile([H, ED], BF16, tag="c16t")
            nc.vector.tensor_copy(out=c16t[:], in_=c16t_ps[:])
            c16t_dram = nc.dram_tensor("c16t_scratch", [H, ED], BF16)
            nc.sync.dma_start(out=c16t_dram[:], in_=c16t[:])
            c16b = cpool.tile([P, H, ED], BF16, tag="c16b")
            nc.sync.dma_start(
                out=c16b[:],
                in_=c16t_dram[:].rearrange("a b -> (a b)")
                    .unsqueeze(0).to_broadcast([P, H * ED]))

            bias_b = cpool.tile([P, HC], F32, tag="bias_b")
            nc.sync.dma_start(out=bias_b[:], in_=bias_d[:].to_broadcast([P, HC]))
            gamma_b = cpool.tile([P, HC], F32, tag="gamma_b")
            nc.sync.dma_start(out=gamma_b[:],
                              in_=gamma_d[:].to_broadcast([P, HC]))
            beta_b = cpool.tile([P, HC], F32, tag="beta_b")
            nc.sync.dma_start(out=beta_b[:], in_=beta_d[:].to_broadcast([P, HC]))

            eps_t = cpool.tile([P, 1], F32, tag="eps_t")
            nc.gpsimd.memset(eps_t[:], LN_EPS)
            tiny_t = cpool.tile([P, 1], F32, tag="tiny_t")
            nc.gpsimd.memset(tiny_t[:], 1e-16)

            # xrb = residual + bias, precomputed once
            xrb_sb = cpool.tile([P, TILES, HC], F32, tag="xrb")
            nc.sync.dma_start(
                out=xrb_sb[:],
                in_=xres_d[:].rearrange("(t p) c -> p t c", p=P))
            nc.vector.tensor_tensor(
                out=xrb_sb[:], in0=xrb_sb[:],
                in1=bias_b[:].unsqueeze(1).to_broadcast([P, TILES, HC]),
                op=OP.add)

            # s_own [128, TILES*H] bf16 (xl . att_src for own nodes)
            xresT_sb = cpool.tile([IN_CH, NPAD], BF16, tag="xresT")
            nc.sync.dma_start(out=xresT_sb[:], in_=xresT_d[:])
            s_own = cpool.tile([P, TILES * H], BF16, tag="s_own")
            for t in range(TILES):
                so_ps = papool.tile([P, H], F32, tag="psA")
                nc.tensor.matmul(out=so_ps[:],
                                 lhsT=xresT_sb[:, t * P:(t + 1) * P],
                                 rhs=rhsBT[:, HC + H:HC + 2 * H],
                                 start=True, stop=True)
                nc.vector.tensor_copy(out=s_own[:, t * H:(t + 1) * H],
                                      in_=so_ps[:])

            # ---------------- phase C: edges (per dst tile) ---------------
            stage_ep = None
            ep_fill = 0
            ep_base = 0

            def flush_epilogue(stage_ep, n_tiles, t0):
                # stage_ep: [P, T_EP, HC+H] f32, tiles t0..t0+n_tiles-1.
                # num cols 0:HC are (c,h)-interleaved; the first multiply
                # below restores standard (h,c) order via a strided read.
                nt = n_tiles
                num = stage_ep[:, :nt, 0:HC]
                den = stage_ep[:, :nt, HC:HC + H]
                rden = epool.tile([P, T_EP, H], F32, tag="rden")
                nc.scalar.activation(out=rden[:, :nt, :], in_=den,
                                     func=AF.Identity, bias=tiny_t[:, 0:1])
                nc.vector.reciprocal(out=rden[:, :nt, :], in_=rden[:, :nt, :])
                o = epool.tile([P, T_EP, HC], F32, tag="o")
                nc.vector.tensor_tensor(
                    out=o[:, :nt, :].rearrange("p t (h c) -> p t h c", c=C),
                    in0=num.rearrange("p t (c h) -> p t h c", h=H),
                    in1=rden[:, :nt, :].unsqueeze(3)
                        .to_broadcast([P, nt, H, C]),
                    op=OP.mult)
                nc.vector.tensor_tensor(out=o[:, :nt, :], in0=o[:, :nt, :],
                                        in1=xrb_sb[:, t0:t0 + nt, :],
                                        op=OP.add)
                # LayerNorm across channels
                mu = epool.tile([P, T_EP], F32, tag="mu")
                nc.vector.reduce_sum(out=mu[:, :nt], in_=o[:, :nt, :],
                                     axis=AX.X)
                nc.scalar.mul(out=mu[:, :nt], in_=mu[:, :nt], mul=1.0 / HC)
                nc.vector.tensor_tensor(
                    out=o[:, :nt, :], in0=o[:, :nt, :],
                    in1=mu[:, :nt].unsqueeze(2).to_broadcast([P, nt, HC]),
                    op=OP.subtract)
                sq = epool.tile([P, T_EP, HC], F32, tag="sq")
                nc.vector.tensor_tensor(out=sq[:, :nt, :], in0=o[:, :nt, :],
                                        in1=o[:, :nt, :], op=OP.mult)
                var = epool.tile([P, T_EP], F32, tag="var")
                nc.vector.reduce_sum(out=var[:, :nt], in_=sq[:, :nt, :],
                                     axis=AX.X)
                # rstd = exp(-0.5 * ln(var/HC + eps))
                nc.scalar.activation(out=var[:, :nt], in_=var[:, :nt],
                                     func=AF.Ln, scale=1.0 / HC,
                                     bias=eps_t[:, 0:1])
                nc.scalar.activation(out=var[:, :nt], in_=var[:, :nt],
                                     func=AF.Exp, scale=-0.5)
                nc.vector.tensor_tensor(
                    out=o[:, :nt, :], in0=o[:, :nt, :],
                    in1=var[:, :nt].unsqueeze(2).to_broadcast([P, nt, HC]),
                    op=OP.mult)
                nc.vector.tensor_tensor(
                    out=o[:, :nt, :], in0=o[:, :nt, :],
                    in1=gamma_b[:].unsqueeze(1).to_broadcast([P, nt, HC]),
                    op=OP.mult)
                nc.vector.tensor_tensor(
                    out=o[:, :nt, :], in0=o[:, :nt, :],
                    in1=beta_b[:].unsqueeze(1).to_broadcast([P, nt, HC]),
                    op=OP.add)
                # ELU = relu(x) + min(exp(x)-1, 0)
                ex = epool.tile([P, T_EP, HC], F32, tag="ex")
                nc.scalar.activation(out=ex[:, :nt, :], in_=o[:, :nt, :],
                                     func=AF.Exp)
                nc.vector.tensor_scalar(out=ex[:, :nt, :], in0=ex[:, :nt, :],
                                        scalar1=-1.0, scalar2=0.0,
                                        op0=OP.add, op1=OP.min)
                nc.scalar.activation(out=o[:, :nt, :], in_=o[:, :nt, :],
                                     func=AF.Relu)
                nc.vector.tensor_tensor(out=o[:, :nt, :], in0=o[:, :nt, :],
                                        in1=ex[:, :nt, :], op=OP.add)
                nc.sync.dma_start(
                    out=out_d[t0 * P:(t0 + nt) * P, :]
                        .rearrange("(t p) c -> p t c", p=P),
                    in_=o[:, :nt, :])

            def emit_acc(msg_t, oh_t, t):
                # accumulate [numerator | denom] for tile t (deferred one
                # tile so the PE never stalls waiting for msg)
                nonlocal stage_ep, ep_fill, ep_base
                acc = pcpool.tile([P, HC + H], F32, name="acc_t", tag="acc")
                for cidx in range(M):
                    nc.tensor.matmul(
                        out=acc[:],
                        lhsT=oh_t[:, cidx, :].bitcast(FP8),
                        rhs=msg_t[:, cidx, :],
                        start=(cidx == 0), stop=(cidx == M - 1))
                if ep_fill == 0:
                    stage_ep = epool.tile([P, T_EP, HC + H], F32,
                                          tag="stage_ep")
                    ep_base = t
                nc.scalar.copy(out=stage_ep[:, t - ep_base, :],
                               in_=acc[:])
                ep_fill += 1
                if ep_fill == T_EP or t == TILES - 1:
                    flush_epilogue(stage_ep, ep_fill, ep_base)
                    ep_fill = 0

            pend = None
            for t in range(TILES):
                cb = t * M

                xjT_sb = spool.tile([IN_CH, M * P], BF16, tag="xjt")
                nc.sync.dma_start(out=xjT_sb[:],
                                  in_=xjT_d[:, cb * P:(cb + M) * P])
                oh_sb = spool.tile([P, M, P], U8, tag="oh")
                nc.sync.dma_start(out=oh_sb[:], in_=oh_d[:, cb:cb + M, :])
                ohdt_sb = spool.tile([P, M, P], U8, tag="ohdt")
                nc.sync.dma_start(out=ohdt_sb[:], in_=ohdt_d[:, cb:cb + M, :])
                ea_sb = spool.tile([P, M, ED], BF16, tag="ea")
                nc.sync.dma_start(out=ea_sb[:], in_=ea_d[:, cb:cb + M, :])

                # ae = (ea * C16t) tree-summed over ED on DVE (bf16, 2x) —
                # emitted first so the DVE starts as soon as ea_sb lands.
                prod = wpool.tile([P, M, H, ED], BF16, tag="prod")
                nc.vector.tensor_tensor(
                    out=prod[:],
                    in0=ea_sb[:].unsqueeze(2).to_broadcast([P, M, H, ED]),
                    in1=c16b[:].unsqueeze(1).to_broadcast([P, M, H, ED]),
                    op=OP.mult)
                for w in (8, 4, 2, 1):
                    nc.vector.tensor_tensor(
                        out=prod[:, :, :, 0:w],
                        in0=prod[:, :, :, 0:w],
                        in1=prod[:, :, :, w:2 * w],
                        op=OP.add)

                # sdst = OHdt^T s_own  (per chunk, 4 cols) — emitted before
                # mm_x so the DVE can fold ae+sdst while ACT copies run.
                al_ps = plpool.tile([P, M * H], F32, tag="al")
                for cidx in range(M):
                    sl = slice(cidx * H, (cidx + 1) * H)
                    nc.tensor.matmul(
                        out=al_ps[:, sl],
                        lhsT=ohdt_sb[:, cidx, :].bitcast(FP8),
                        rhs=s_own[:, t * H:(t + 1) * H],
                        start=True, stop=True, skip_group_check=True)

                # alpha-partial = ae + sdst (aj not ready yet)
                alpha = wpool.tile([P, M, H], BF16, tag="alpha")
                nc.vector.tensor_tensor(
                    out=alpha[:],
                    in0=prod[:, :, :, 0],
                    in1=al_ps[:].rearrange("p (m h) -> p m h", h=H),
                    op=OP.add)

                # xl_j | aj per chunk via PE (already (c,h)-interleaved —
                # W_lin rows were permuted host-side), one merged ACT copy.
                xj_sb = wpool.tile([P, M, HC + H], BF16, tag="xj")
                for g0 in range(0, M, G3):
                    k = min(G3, M - g0)
                    psx = pxpool.tile([P, G3, HC + H], F32, tag="psx")
                    for i in range(k):
                        nc.tensor.matmul(
                            out=psx[:, i, :],
                            lhsT=xjT_sb[:, (g0 + i) * P:(g0 + i + 1) * P],
                            rhs=rhsBT[:, 0:HC + H],
                            start=True, stop=True)
                    nc.scalar.copy(out=xj_sb[:, g0:g0 + k, :],
                                   in_=psx[:, :k, :])

                # previous tile's accumulation goes here on the PE queue:
                # its msg is ready by now, and it fills the PE pipeline
                # while this tile's alpha/exp/msg run on ACT/DVE.
                if pend is not None:
                    emit_acc(*pend)

                # alpha = lrelu(alpha-partial + aj)
                nc.vector.tensor_tensor(
                    out=alpha[:],
                    in0=alpha[:],
                    in1=xj_sb[:, :, HC:HC + H],
                    op=OP.add)
                nc.vector.scalar_tensor_tensor(
                    out=alpha[:], in0=alpha[:],
                    scalar=NEG_SLOPE, in1=alpha[:],
                    op0=OP.mult, op1=OP.max)

                # msg = e * xl  ((c,h)-interleaved, DVE 2x bf16 mode)
                msg = wpool.tile([P, M, HC + H], BF16, tag="msg")
                nc.scalar.activation(out=msg[:, :, HC:HC + H],
                                     in_=alpha[:], func=AF.Exp)
                nc.vector.tensor_tensor(
                    out=msg[:, :, 0:HC].rearrange("p m (c h) -> p m c h", h=H),
                    in0=xj_sb[:, :, 0:HC]
                        .rearrange("p m (c h) -> p m c h", h=H),
                    in1=msg[:, :, HC:HC + H].unsqueeze(2)
                        .to_broadcast([P, M, C, H]),
                    op=OP.mult)

                pend = (msg, oh_sb, t)
            emit_acc(*pend)

    nc.compile()
    return nc


# --------------------------------------------------------------------------
# entry point
# --------------------------------------------------------------------------

def kernel(**inputs) -> np.ndarray:
    in_maps, M = host_prep(**inputs)
    nc = build_program(M)
    res = run_bass_kernel_spmd(nc, in_maps, list(range(N_CORES)))
    parts = [res.results[c]["out"][:NPC] for c in range(N_CORES)]
    return np.concatenate(parts, axis=0).astype(np.float32)
